# revision 4
# baseline (speedup 1.0000x reference)
"""Bass/Trainium2 kernel for nn_BMGAE (LightGCN-style 2-layer propagation on
three bipartite graphs), sharded across 8 NeuronCores.

Strategy:
  - For each graph (n nodes), host assigns nodes to cores round-robin by
    degree rank (balances per-core edge counts); each core owns a padded
    slice of node rows.
  - Host buckets/sorts each core's edges by (owner src block, src); pads
    per-block runs to a common length across cores (SPMD: one program).
  - Device, per layer: gathers cur[dst] rows (SWDGE), builds a one-hot
    [slots x 128] matrix from src-local ids (DVE is_equal vs iota), and
    accumulates per-block node sums with TensorE matmuls into PSUM.
    Epilogue per block-group: scale 1/(l+2), L2-normalize, accumulate acc.
  - AllGather shares the full cur_1 table for layer-2 gathers.
  - Host reassembles + unpermutes the final [220000, 64] output.

kernel(**inputs) takes the FULL unsharded inputs and returns the FULL output.
"""
import numpy as np

import concourse.tile as tile
from concourse import bass, bacc, mybir
from concourse.bass_utils import run_bass_kernel_spmd

P = 128
N_CORES = 8
D = 64
EPS_NORM = 1e-12
LAYER_NUM = 2
B_PP = 8          # blocks per epilogue batch
GQ = 4            # SWDGE queues
CHUNK = 32768     # int16 dma_gather index range per table chunk
MAX_NI = 1024     # max rows per dma_gather instruction
COMPUTE_DT = "fp32"  # "fp32" | "fp16" for the segment-sum matmul operands

# graph definitions: (name, leftkey, rightkey, srckey, dstkey, valkey)
GRAPHS = [
    ("ui", "users", "items", "ui_src", "ui_dst", "ui_val"),
    ("ub", "users", "bundles", "ub_src", "ub_dst", "ub_val"),
    ("bi", "bundles", "items", "bi_src", "bi_dst", "bi_val"),
]


def _ceil(a, b):
    return -(-a // b)


class GraphPlan:
    """Host-side plan for one graph: permutation, padded runs, index arrays."""

    def __init__(self, name, n, src, dst, val):
        self.name = name
        self.n = n
        # node -> (core, slot) balanced by degree
        deg = np.bincount(src, minlength=n)
        order = np.argsort(-deg, kind="stable")   # rank -> node
        rank = np.empty(n, dtype=np.int64)
        rank[order] = np.arange(n)
        self.core_of = (rank % N_CORES).astype(np.int64)
        j = rank // N_CORES  # rank within core
        self.n_slice = _ceil(n, N_CORES)
        self.n_slice_pad = _ceil(self.n_slice, P) * P
        self.blocks = self.n_slice_pad // P
        # stratify degrees across blocks: deal consecutive degree-ranks
        # round-robin over blocks so per-block edge counts are flat
        self.slot_of = (j % self.blocks) * P + j // self.blocks
        self.n_pad = self.n_slice_pad * N_CORES
        # global padded id (row in the cur table)
        self.gid_of = self.core_of * self.n_slice_pad + self.slot_of

        dst_g = self.gid_of[dst]
        src_core = self.core_of[src]
        src_slot = self.slot_of[src]

        # choose the chunk size (int16 range cap 32768) minimizing modeled
        # SWDGE cost: slots * 2.15ns + gather-instructions * 900ns
        best = None
        for nch in range(_ceil(self.n_pad, CHUNK), _ceil(self.n_pad, CHUNK) + 4):
            crows = min(_ceil(_ceil(self.n_pad, nch), P) * P, CHUNK)
            nck = _ceil(self.n_pad, crows)
            counts = np.zeros((N_CORES, self.blocks, nck), dtype=np.int64)
            for k in range(N_CORES):
                m = src_core == k
                np.add.at(counts[k], (src_slot[m] // P, dst_g[m] // crows), 1)
            tmax = counts.max(axis=0)
            run_len = _ceil(np.maximum(tmax, 0), P) * P
            run_len[tmax == 0] = 0
            slots = int(run_len.sum())
            pieces = int(np.ceil(run_len[run_len > 0] / MAX_NI).sum())
            cost = slots * 2.15 + pieces * 900.0
            if best is None or cost < best[0]:
                best = (cost, crows, nck, run_len)
        _, crows, nck, run_len = best
        self.chunk = crows
        self.nchunks = nck
        self.run_len = run_len

        # per-core sorted edge lists, bucketed by (block, chunk)
        per_core = []
        for k in range(N_CORES):
            m = src_core == k
            ss, dd, vv = src_slot[m], dst_g[m], val[m]
            blk = ss // P
            ch = dd // crows
            o = np.lexsort((dd, ss, ch, blk))
            per_core.append((blk[o], ch[o], ss[o], dd[o], vv[o]))

        self.total_slots = int(self.run_len.sum())
        self.total_tiles = self.total_slots // P
        # tile offset of each (block, chunk) run
        self.run_tile_off = np.zeros((self.blocks, self.nchunks), dtype=np.int64)
        t = 0
        for b in range(self.blocks):
            for c in range(self.nchunks):
                self.run_tile_off[b, c] = t
                t += self.run_len[b, c] // P
        # per-block tile range
        self.block_tile_off = self.run_tile_off[:, 0].copy()
        self.block_tiles = (self.run_len.sum(axis=1) // P).astype(np.int64)

        # build per-core arrays: idx32 (global dst), idx16 (chunk-local,
        # wrapped), srcrel f32, val f32
        self.idx32 = np.zeros((N_CORES, P, self.total_tiles), dtype=np.int32)
        self.idx16 = np.zeros((N_CORES, P, self.total_slots // 16), dtype=np.int16)
        self.srcrel = np.full((N_CORES, P, self.total_tiles), -1.0, dtype=np.float16)
        self.valar = np.zeros((N_CORES, P, self.total_tiles), dtype=np.float32)
        for k in range(N_CORES):
            blk, ch, ss, dd, vv = per_core[k]
            # positions of this core's edges inside the padded runs
            # edges are sorted by (blk, ch, ...) already
            idx_flat = np.zeros(self.total_slots, dtype=np.int64)  # global dst id
            srcrel_flat = np.full(self.total_slots, -1.0, dtype=np.float16)
            val_flat = np.zeros(self.total_slots, dtype=np.float32)
            # slot base of each run
            run_slot_off = self.run_tile_off * P
            # group edges by run
            pos = 0
            e0 = 0
            key = blk * self.nchunks + ch
            bounds = np.searchsorted(key, np.arange(self.blocks * self.nchunks + 1))
            for b in range(self.blocks):
                for c in range(self.nchunks):
                    kk = b * self.nchunks + c
                    lo, hi = bounds[kk], bounds[kk + 1]
                    L = self.run_len[b, c]
                    if L == 0:
                        assert hi == lo
                        continue
                    base = run_slot_off[b, c]
                    cnt = hi - lo
                    assert cnt <= L
                    idx_flat[base:base + cnt] = dd[lo:hi]
                    # pad slots keep idx pointing inside this chunk
                    idx_flat[base + cnt:base + L] = c * self.chunk
                    srcrel_flat[base:base + cnt] = (ss[lo:hi] - b * P).astype(np.float16)
                    val_flat[base:base + cnt] = vv[lo:hi]
            # arrange: slot j -> (partition j%P, tile j//P)
            self.idx32[k] = idx_flat.reshape(self.total_tiles, P).T.astype(np.int32)
            self.srcrel[k] = srcrel_flat.reshape(self.total_tiles, P).T
            self.valar[k] = val_flat.reshape(self.total_tiles, P).T
            # int16 wrapped: per run, chunk-local idx k -> partition k%16, col k//16,
            # replicated x8 down partitions
            loc = idx_flat.copy()
            for b in range(self.blocks):
                for c in range(self.nchunks):
                    L = self.run_len[b, c]
                    if L == 0:
                        continue
                    base = run_slot_off[b, c]
                    loc[base:base + L] -= c * self.chunk
            assert loc.min() >= 0 and loc.max() < 32768
            w = loc.reshape(self.total_slots // 16, 16).T.astype(np.int16)  # [16, S/16]
            self.idx16[k] = np.tile(w, (8, 1))

    def make_table(self, left, right):
        """Build padded+permuted cur0 table [n_pad, D] from the two halves."""
        reps = np.concatenate([left, right], axis=0).astype(np.float32)
        tab = np.zeros((self.n_pad, D), dtype=np.float32)
        tab[self.gid_of] = reps
        return tab

    def unpermute(self, acc_slices):
        """acc_slices: [N_CORES, n_slice_pad, D] -> natural-order [n, D]."""
        full = np.concatenate(acc_slices, axis=0)  # [n_pad, D] in gid order
        return full[self.gid_of]


def build_program(plans, use_dma_gather=True):
    nc = bacc.Bacc("TRN2", target_bir_lowering=False, debug=False,
                   num_devices=N_CORES, num_swdge_queues=GQ)

    # ---- declare I/O ----
    tabs, idxs, srcs, vals = {}, {}, {}, {}
    for gp in plans:
        tabs[gp.name] = nc.declare_dram_parameter(
            f"tab_{gp.name}", [gp.n_pad, D], mybir.dt.float32, isOutput=False)
        if use_dma_gather:
            idxs[gp.name] = nc.declare_dram_parameter(
                f"idx_{gp.name}", [P, gp.total_slots // 16], mybir.dt.int16,
                isOutput=False)
        else:
            idxs[gp.name] = nc.declare_dram_parameter(
                f"idx_{gp.name}", [P, gp.total_tiles], mybir.dt.int32,
                isOutput=False)
        srcs[gp.name] = nc.declare_dram_parameter(
            f"srcrel_{gp.name}", [P, gp.total_tiles], mybir.dt.float16,
            isOutput=False)
        vals[gp.name] = nc.declare_dram_parameter(
            f"val_{gp.name}", [P, gp.total_tiles], mybir.dt.float32,
            isOutput=False)
    out_rows = sum(gp.n_slice_pad for gp in plans)
    # acc tensors are stored partition-major: [P, (rows/P) * D]
    out_blocks = out_rows // P
    reps_own = nc.declare_dram_parameter(
        "reps_own", [P, out_blocks * D], mybir.dt.float32, isOutput=False)
    iota_in = nc.declare_dram_parameter(
        "iota", [P, P], mybir.dt.float16, isOutput=False)
    acc_out = nc.declare_dram_parameter(
        "acc_out", [P, out_blocks * D], mybir.dt.float32, isOutput=True)

    # internal DRAM
    acc1 = nc.dram_tensor("acc1", [P, out_blocks * D], mybir.dt.float32)
    ag_in, ag_out = {}, {}
    for gp in plans:
        ag_in[gp.name] = nc.dram_tensor(
            f"ag_in_{gp.name}", [gp.n_slice_pad, D], mybir.dt.float32)
        ag_out[gp.name] = nc.dram_tensor(
            f"ag_out_{gp.name}", [gp.n_pad, D], mybir.dt.float32,
            addr_space="Shared")

    gq_counter = [0]

    with tile.TileContext(nc) as tc:
        with tc.tile_pool(name="const", bufs=1) as constp, \
             tc.tile_pool(name="meta", bufs=10) as metap, \
             tc.tile_pool(name="gpool", bufs=14) as gpool, \
             tc.tile_pool(name="wpool", bufs=8) as wpool, \
             tc.tile_pool(name="stg", bufs=4) as stgp, \
             tc.tile_pool(name="post", bufs=2) as postp, \
             tc.tile_pool(name="psum", bufs=8, space="PSUM") as psump:

            iota_t = constp.tile([P, P], mybir.dt.float16)
            nc.sync.dma_start(out=iota_t[:], in_=iota_in[:, :])

            def do_graph_layer(gp, layer, table, acc_prev, acc_next, cur_out):
                """Emit one propagation layer for one graph.

                table: DRAM [n_pad, D] to gather from
                acc_prev: DRAM [n_slice_pad, D] slice (previous acc), or None->reps
                acc_next: DRAM [n_slice_pad, D] slice to write acc into
                cur_out: DRAM [n_slice_pad, D] to write cur_next into (or None)
                """
                inv = 1.0 / (layer + 2)
                nblocks = gp.blocks
                ngroups = _ceil(nblocks, B_PP)
                for grp in range(ngroups):
                    b0 = grp * B_PP
                    b1 = min(b0 + B_PP, nblocks)
                    nb = b1 - b0
                    stg = stgp.tile([P, B_PP * D], mybir.dt.float32, tag="stg")
                    for b in range(b0, b1):
                        tb = int(gp.block_tiles[b])
                        if tb == 0:
                            # no edges: block sums are zero
                            nc.vector.memset(
                                stg[:, (b - b0) * D:(b - b0 + 1) * D], 0.0)
                            continue
                        t0 = int(gp.block_tile_off[b])
                        # load meta arrays for this block
                        sr = metap.tile([P, tb], mybir.dt.float16, tag="sr")
                        vl = metap.tile([P, tb], mybir.dt.float32, tag="vl")
                        nc.sync.dma_start(out=sr[:], in_=srcs[gp.name][:, t0:t0 + tb])
                        nc.sync.dma_start(out=vl[:], in_=vals[gp.name][:, t0:t0 + tb])
                        ps = psump.tile([P, 2 * D], mybir.dt.float32, tag="ps")
                        # per-gather pieces: gather -> scale -> one-hot -> matmuls
                        MAXT = MAX_NI // P
                        pieces = []  # (chunk, tile_off_in_graph, ntiles)
                        for c in range(gp.nchunks):
                            L = int(gp.run_len[b, c])
                            if L == 0:
                                continue
                            roff = int(gp.run_tile_off[b, c])
                            lt = L // P
                            off = 0
                            while off < lt:
                                sz = min(MAXT, lt - off)
                                pieces.append((c, roff + off, sz))
                                off += sz
                        npieces = len(pieces)
                        tdone = 0
                        for ip, (c, toff, nt) in enumerate(pieces):
                            ni = nt * P
                            so = toff * P
                            bt = toff - t0  # tile offset within block
                            g = gpool.tile([P, MAXT * D], mybir.dt.float32,
                                           tag="g")
                            if use_dma_gather:
                                cbase = c * gp.chunk
                                csz = min(gp.chunk, gp.n_pad - cbase)
                                it = metap.tile([P, MAX_NI // 16],
                                                mybir.dt.int16, tag="idx")
                                nc.sync.dma_start(
                                    out=it[:, :ni // 16],
                                    in_=idxs[gp.name][:, so // 16:(so + ni) // 16])
                                nc.gpsimd.dma_gather(
                                    g[:, :nt * D]
                                        .rearrange("p (t d) -> p t d", d=D),
                                    table[cbase:cbase + csz, :],
                                    it[:, :ni // 16],
                                    ni, ni, D,
                                    queue_num=gq_counter[0] % GQ,
                                )
                                gq_counter[0] += 1
                            else:
                                idx_t = metap.tile([P, MAXT], mybir.dt.int32,
                                                   tag="idx")
                                nc.sync.dma_start(
                                    out=idx_t[:, :nt],
                                    in_=idxs[gp.name][:, toff:toff + nt])
                                for t in range(nt):
                                    nc.gpsimd.indirect_dma_start(
                                        out=g[:, t * D:(t + 1) * D],
                                        out_offset=None,
                                        in_=table[:, :],
                                        in_offset=bass.IndirectOffsetOnAxis(
                                            ap=idx_t[:, t:t + 1], axis=0))
                            # g2f = val * G in fp32, then split into exact
                            # fp16 hi/lo halves: g2cat = [hi | lo] per tile.
                            # One fp16 matmul per tile (1 cyc/row vs 4 for
                            # fp32) computes both halves; epilogue sums them.
                            cdt = mybir.dt.float16
                            g2f = gpool.tile([P, MAXT * D], mybir.dt.float32,
                                             tag="g2f")
                            nc.vector.tensor_tensor(
                                out=g2f[:, :nt * D]
                                    .rearrange("p (t d) -> p t d", d=D),
                                in0=g[:, :nt * D]
                                    .rearrange("p (t d) -> p t d", d=D),
                                in1=vl[:, bt:bt + nt].to_broadcast([P, nt, D]),
                                op=mybir.AluOpType.mult)
                            g2c = gpool.tile([P, MAXT * 2 * D], cdt,
                                             tag="g2c")
                            nc.scalar.mul(
                                g2c[:, :nt * 2 * D]
                                    .rearrange("p (t d) -> p t d", d=2 * D)
                                    [:, :, 0:D],
                                g2f[:, :nt * D]
                                    .rearrange("p (t d) -> p t d", d=D),
                                1.0)
                            g2h = gpool.tile([P, MAXT * D], mybir.dt.float32,
                                             tag="g2h")
                            nc.scalar.mul(
                                g2h[:, :nt * D]
                                    .rearrange("p (t d) -> p t d", d=D),
                                g2c[:, :nt * 2 * D]
                                    .rearrange("p (t d) -> p t d", d=2 * D)
                                    [:, :, 0:D],
                                1.0)
                            lo32 = gpool.tile([P, MAXT * D],
                                              mybir.dt.float32, tag="lo32")
                            nc.vector.tensor_tensor(
                                out=lo32[:, :nt * D]
                                    .rearrange("p (t d) -> p t d", d=D),
                                in0=g2f[:, :nt * D]
                                    .rearrange("p (t d) -> p t d", d=D),
                                in1=g2h[:, :nt * D]
                                    .rearrange("p (t d) -> p t d", d=D),
                                op=mybir.AluOpType.subtract)
                            # scale residual into fp16 normal range (else it
                            # is subnormal and flushes to zero in the PE)
                            nc.scalar.mul(
                                g2c[:, :nt * 2 * D]
                                    .rearrange("p (t d) -> p t d", d=2 * D)
                                    [:, :, D:2 * D],
                                lo32[:, :nt * D]
                                    .rearrange("p (t d) -> p t d", d=D),
                                2048.0)
                            # one-hot W for this piece
                            w = wpool.tile([P, MAXT * P], cdt, tag="w")
                            nc.vector.tensor_tensor(
                                out=w[:, :nt * P]
                                    .rearrange("p (t q) -> p t q", q=P),
                                in0=sr[:, bt:bt + nt].to_broadcast([P, nt, P]),
                                in1=iota_t[:, None, :].to_broadcast([P, nt, P]),
                                op=mybir.AluOpType.is_equal)
                            for t in range(nt):
                                nc.tensor.matmul(
                                    out=ps[:],
                                    lhsT=w[:, t * P:(t + 1) * P],
                                    rhs=g2c[:, t * 2 * D:(t + 1) * 2 * D],
                                    start=(tdone + t == 0),
                                    stop=(tdone + t == tb - 1))
                            tdone += nt
                        # cur_next = (ps_hi + ps_lo) * inv -> staging column
                        col = stg[:, (b - b0) * D:(b - b0 + 1) * D]
                        pst = postp.tile([P, D], mybir.dt.float32, tag="pst")
                        nc.scalar.mul(col, ps[:, 0:D], inv)
                        nc.scalar.mul(pst[:], ps[:, D:2 * D], inv / 2048.0)
                        nc.vector.tensor_tensor(
                            out=col, in0=col, in1=pst[:],
                            op=mybir.AluOpType.add)
                    # ---- epilogue for this block group ----
                    sq = postp.tile([P, B_PP * D], mybir.dt.float32, tag="sq")
                    nc.vector.tensor_tensor(
                        out=sq[:, :nb * D], in0=stg[:, :nb * D],
                        in1=stg[:, :nb * D], op=mybir.AluOpType.mult)
                    ssum = postp.tile([P, B_PP], mybir.dt.float32, tag="ssum")
                    nc.vector.tensor_reduce(
                        out=ssum[:, :nb],
                        in_=sq[:, :nb * D].rearrange("p (b d) -> p b d", d=D),
                        axis=mybir.AxisListType.X,
                        op=mybir.AluOpType.add)
                    nrm = postp.tile([P, B_PP], mybir.dt.float32, tag="nrm")
                    nc.scalar.activation(out=nrm[:, :nb], in_=ssum[:, :nb],
                                         func=mybir.ActivationFunctionType.Sqrt)
                    nc.vector.tensor_scalar_max(
                        out=nrm[:, :nb], in0=nrm[:, :nb], scalar1=EPS_NORM)
                    rec = postp.tile([P, B_PP], mybir.dt.float32, tag="rec")
                    nc.vector.reciprocal(out=rec[:, :nb], in_=nrm[:, :nb])
                    normed = postp.tile([P, B_PP * D], mybir.dt.float32, tag="nd")
                    nc.vector.tensor_tensor(
                        out=normed[:, :nb * D].rearrange("p (b d) -> p b d", d=D),
                        in0=stg[:, :nb * D].rearrange("p (b d) -> p b d", d=D),
                        in1=rec[:, :nb].to_broadcast([P, nb, D]),
                        op=mybir.AluOpType.mult)
                    # acc_next = acc_prev + normed (partition-major layout)
                    prev = postp.tile([P, B_PP * D], mybir.dt.float32, tag="pv")
                    nc.sync.dma_start(
                        out=prev[:, :nb * D],
                        in_=acc_prev[:, b0 * D:(b0 + nb) * D])
                    accn = postp.tile([P, B_PP * D], mybir.dt.float32, tag="an")
                    nc.vector.tensor_tensor(
                        out=accn[:, :nb * D], in0=prev[:, :nb * D],
                        in1=normed[:, :nb * D], op=mybir.AluOpType.add)
                    nc.sync.dma_start(
                        out=acc_next[:, b0 * D:(b0 + nb) * D],
                        in_=accn[:, :nb * D])
                    if cur_out is not None:
                        nc.sync.dma_start(
                            out=cur_out[b0 * P:b0 * P + nb * P, :]
                                .rearrange("(b p) d -> p b d", p=P),
                            in_=stg[:, :nb * D].rearrange("p (b d) -> p b d", d=D))

            # ---- layer 1 (each graph's AllGather fires as soon as its
            # layer-1 finishes, overlapping the remaining compute) ----
            blk0 = 0
            for gp in plans:
                c0, c1 = blk0 * D, (blk0 + gp.blocks) * D
                do_graph_layer(
                    gp, 0,
                    table=tabs[gp.name],
                    acc_prev=reps_own[:, c0:c1],
                    acc_next=acc1[:, c0:c1],
                    cur_out=ag_in[gp.name])
                blk0 += gp.blocks
                nc.gpsimd.collective_compute(
                    "AllGather",
                    mybir.AluOpType.bypass,
                    ins=[ag_in[gp.name][:, :]],
                    outs=[ag_out[gp.name][:, :]],
                    replica_groups=[list(range(N_CORES))],
                )

            # ---- layer 2 ----
            blk0 = 0
            for gp in plans:
                c0, c1 = blk0 * D, (blk0 + gp.blocks) * D
                do_graph_layer(
                    gp, 1,
                    table=ag_out[gp.name],
                    acc_prev=acc1[:, c0:c1],
                    acc_next=acc_out[:, c0:c1],
                    cur_out=None)
                blk0 += gp.blocks

    nc.compile()
    return nc


def _run(inputs, use_dma_gather=True, trace=False):
    users = np.asarray(inputs["users"], dtype=np.float32)
    bundles = np.asarray(inputs["bundles"], dtype=np.float32)
    items = np.asarray(inputs["items"], dtype=np.float32)
    halves = {"ui": (users, items), "ub": (users, bundles), "bi": (bundles, items)}

    plans = []
    for name, lk, rk, sk, dk, vk in GRAPHS:
        n = inputs[lk].shape[0] + inputs[rk].shape[0]
        plans.append(GraphPlan(
            name, n,
            np.asarray(inputs[sk]), np.asarray(inputs[dk]),
            np.asarray(inputs[vk], dtype=np.float32)))

    nc = build_program(plans, use_dma_gather=use_dma_gather)

    iota = np.tile(np.arange(P, dtype=np.float16)[None, :], (P, 1))
    in_maps = []
    for k in range(N_CORES):
        m = {"iota": iota}
        reps_parts = []
        for gp in plans:
            tab = gp.make_table(*halves[gp.name])
            m[f"tab_{gp.name}"] = tab
            m[f"idx_{gp.name}"] = gp.idx16[k] if use_dma_gather else gp.idx32[k]
            m[f"srcrel_{gp.name}"] = gp.srcrel[k]
            m[f"val_{gp.name}"] = gp.valar[k]
            reps_parts.append(
                tab[k * gp.n_slice_pad:(k + 1) * gp.n_slice_pad])
        # partition-major: [P, blocks*D]
        pm = [r.reshape(-1, P, D).transpose(1, 0, 2).reshape(P, -1)
              for r in reps_parts]
        m["reps_own"] = np.ascontiguousarray(np.concatenate(pm, axis=1))
        in_maps.append(m)

    res = run_bass_kernel_spmd(nc, in_maps, list(range(N_CORES)), trace=trace)

    # reassemble (acc_out is partition-major [P, blocks*D])
    acc = {}
    blk0 = 0
    for gp in plans:
        slices = []
        for k in range(N_CORES):
            a = res.results[k]["acc_out"][:, blk0 * D:(blk0 + gp.blocks) * D]
            a = a.reshape(P, gp.blocks, D).transpose(1, 0, 2).reshape(-1, D)
            slices.append(a)
        acc[gp.name] = gp.unpermute(np.stack(slices))
        blk0 += gp.blocks

    NU, NB, NI_ = users.shape[0], bundles.shape[0], items.shape[0]
    il_u, il_i = acc["ui"][:NU], acc["ui"][NU:]
    bl_u, bl_b = acc["ub"][:NU], acc["ub"][NU:]
    bs_b, bs_i = acc["bi"][:NB], acc["bi"][NB:]
    out = np.concatenate([il_u, bl_u, bl_b, bs_b, il_i, bs_i], axis=0)
    return out, res


def kernel(**inputs) -> np.ndarray:
    out, _ = _run(inputs, use_dma_gather=True, trace=False)
    return out

